# revision 4
# baseline (speedup 1.0000x reference)
"""Trainium2 Bass kernel for nn_Attention_40492951666725.

Full attention layer: qkv proj -> RoPE (interleaved pairs, rot dim 32) ->
softmax(QK^T)V -> out proj.  B=4, N=2048, DIM=1024, H=16, DH=64.

Sharding: 8 cores, core c handles batch b=c//2 and query-half c%2 (1024
query tokens, all 16 heads, full 2048-token K/V).  K/V projection is
computed redundantly by the two cores sharing a batch; no collectives.
The host rotates the token axis per core so the core's own query tokens
are always columns [0:1024] of xT (attention is permutation-invariant
over keys, so k/v/cos/sin just follow the same order).

Layouts (per core):
  xT   [DIM, 2048]  (host-transposed)   -> lhsT/rhs for projections
  q^T  [feat, 1024], k^T [feat, 2048]   feat on partitions
  S^T  [kj, qi]  (kj on partitions)     -> softmax via exp (no max-sub;
        scores are O(+-10) so fp32 exp is safe), denominator from a
        ones-column appended to V (M=65 AV matmuls), division applied to
        the [64, qi] head output (commutes with the PV sum).
  attn^T [inner, tok] -> out proj produces out [tok, DIM] directly.

RoPE: rotate_every_two(q) is a fixed feat-space linear map -> done with a
single [128,128] block-diagonal matmul (Rm), then q_rot = q*cos + (Rq)*sin
elementwise on DVE; pass-dims use cos=1/sin=0 so all 64 dims are uniform.
"""

import os
import numpy as np
import ml_dtypes

import concourse.bass as bass
from concourse import bacc
import concourse.tile as tile
from concourse import mybir, library_config
from concourse.bass_utils import run_bass_kernel_spmd

BF = ml_dtypes.bfloat16
bf16 = mybir.dt.bfloat16
f32 = mybir.dt.float32

B, N, DIM, H, DH, ROT = 4, 2048, 1024, 16, 64, 32
INNER = H * DH
NQ = N // 2            # query tokens per core
NCORES = 8
P = 128
KD = DIM // P          # 8 contraction tiles over model dim
NKT = N // P           # 16 kj partition tiles
HPB = H // 2           # 8 head-pair blocks

Exp = mybir.ActivationFunctionType.Exp

_CACHE = {}


def _build_rope_consts():
    """cos_pad/sin_pad [128, N] for one head-pair feat block, Rm [128,128]."""
    pos = np.arange(N, dtype=np.float64)[:, None]
    inv_freq = 1.0 / (10000.0 ** (np.arange(0, ROT, 2, dtype=np.float64) / ROT))
    ang = np.repeat(pos * inv_freq, 2, axis=-1)          # [N, ROT]
    sin, cos = np.sin(ang), np.cos(ang)                  # [N, 32]

    cos_pad = np.ones((P, N), np.float32)
    sin_pad = np.zeros((P, N), np.float32)
    for half in range(2):                                # two heads per block
        r0 = half * DH
        cos_pad[r0:r0 + ROT, :] = cos.T
        sin_pad[r0:r0 + ROT, :] = sin.T

    # Rm[dp, d]: out[d] = sum_dp Rm[dp, d] * q[dp]  == rotate_every_two(q)[d]
    Rm = np.zeros((P, P), np.float32)
    for half in range(2):
        r0 = half * DH
        for i in range(0, ROT, 2):
            Rm[r0 + i + 1, r0 + i] = -1.0                # out[2i]   = -q[2i+1]
            Rm[r0 + i, r0 + i + 1] = 1.0                 # out[2i+1] =  q[2i]
    return cos_pad, sin_pad, Rm


def _build_program():
    nc = bacc.Bacc(trn_type="TRN2")

    xkv_d = nc.dram_tensor("xkv", [DIM, N], bf16, kind="ExternalInput")
    wq_d = nc.dram_tensor("wq", [DIM, INNER], bf16, kind="ExternalInput")
    wk_d = nc.dram_tensor("wk", [DIM, INNER], bf16, kind="ExternalInput")
    wv_d = nc.dram_tensor("wv", [DIM, INNER], bf16, kind="ExternalInput")
    wo_d = nc.dram_tensor("wo", [INNER, DIM], bf16, kind="ExternalInput")
    cosk_d = nc.dram_tensor("cosk", [P, N], bf16, kind="ExternalInput")
    sink_d = nc.dram_tensor("sink", [P, N], bf16, kind="ExternalInput")
    rm_d = nc.dram_tensor("rm", [P, P], bf16, kind="ExternalInput")
    out_d = nc.dram_tensor("out", [NQ, DIM], f32, kind="ExternalOutput")

    with tile.TileContext(nc) as tc:
        with (
            tc.tile_pool(name="res", bufs=1) as res,          # kernel-lifetime tiles
            tc.tile_pool(name="kstream", bufs=2) as kstream,  # per-hp q/k tiles
            tc.tile_pool(name="vstream", bufs=2) as vstream,  # v_aug per-hp x16
            tc.tile_pool(name="pt", bufs=4) as ptp,           # P^T tiles
            tc.tile_pool(name="tmp", bufs=2) as tmp,          # rope DVE temps
            tc.tile_pool(name="small", bufs=4) as small,
            tc.tile_pool(name="ostage", bufs=2) as ostage,
            tc.tile_pool(name="psA", bufs=2, space="PSUM") as psA,    # [128,512] proj/outproj/swap
            tc.tile_pool(name="psS", bufs=2, space="PSUM") as psS,    # [128,1024] scores
            tc.tile_pool(name="psV", bufs=2, space="PSUM") as psV,    # [65,512] AV
        ):
            nc.gpsimd.load_library(library_config.attn)

            # ---- resident loads ----
            xkv, wq, wk, wv, wo = [], [], [], [], []
            for k in range(KD):
                t = res.tile([P, N], bf16, tag=f"xkv{k}", name=f"xkv{k}")
                nc.sync.dma_start(t[:], xkv_d[k * P:(k + 1) * P, :])
                xkv.append(t)
                for nm, lst, dram in (("wq", wq, wq_d), ("wk", wk, wk_d),
                                      ("wv", wv, wv_d), ("wo", wo, wo_d)):
                    t = res.tile([P, DIM], bf16, tag=f"{nm}{k}", name=f"{nm}{k}")
                    nc.sync.dma_start(t[:], dram[k * P:(k + 1) * P, :])
                    lst.append(t)
            cosk = res.tile([P, N], bf16, tag="cosk")
            sink = res.tile([P, N], bf16, tag="sink")
            rm = res.tile([P, P], bf16, tag="rm")
            for t, d in ((cosk, cosk_d), (sink, sink_d), (rm, rm_d)):
                nc.sync.dma_start(t[:], d[:])

            attnT = []
            for k in range(KD):
                attnT.append(res.tile([P, NQ], bf16, tag=f"attnT{k}", name=f"attnT{k}"))

            state = {}

            def emit_proj(hp):
                """Project+rope feat block hp (heads 2hp, 2hp+1)."""
                c0 = hp * P
                # --- q^T block: [128 feats, NQ]  (q tokens = xkv cols 0:NQ) ---
                qraw = kstream.tile([P, NQ], bf16, tag="qraw")
                for n in range(NQ // 512):
                    ps = psA.tile([P, 512], f32, tag="ps")
                    for k in range(KD):
                        nc.tensor.matmul(ps[:], wq[k][:, c0:c0 + P],
                                         xkv[k][:, n * 512:(n + 1) * 512],
                                         start=(k == 0), stop=(k == KD - 1))
                    nc.vector.tensor_copy(qraw[:, n * 512:(n + 1) * 512], ps[:])
                qrot = kstream.tile([P, NQ], bf16, tag="qrot")
                for n in range(NQ // 512):
                    sl = slice(n * 512, (n + 1) * 512)
                    psw = psA.tile([P, 512], f32, tag="ps")
                    nc.tensor.matmul(psw[:], rm[:], qraw[:, sl], start=True, stop=True)
                    t1 = tmp.tile([P, 512], f32, tag="t1")
                    nc.vector.tensor_mul(t1[:], qraw[:, sl], cosk[:, sl])
                    t2 = tmp.tile([P, 512], f32, tag="t2")
                    nc.vector.tensor_mul(t2[:], psw[:], sink[:, sl])
                    nc.vector.tensor_add(qrot[:, sl], t1[:], t2[:])
                # --- k^T block: [128 feats, N] ---
                kraw = kstream.tile([P, N], bf16, tag="kraw")
                for n in range(N // 512):
                    ps = psA.tile([P, 512], f32, tag="ps")
                    for k in range(KD):
                        nc.tensor.matmul(ps[:], wk[k][:, c0:c0 + P],
                                         xkv[k][:, n * 512:(n + 1) * 512],
                                         start=(k == 0), stop=(k == KD - 1))
                    nc.vector.tensor_copy(kraw[:, n * 512:(n + 1) * 512], ps[:])
                krot = kstream.tile([P, N], bf16, tag="krot")
                for n in range(N // 512):
                    sl = slice(n * 512, (n + 1) * 512)
                    psw = psA.tile([P, 512], f32, tag="ps")
                    nc.tensor.matmul(psw[:], rm[:], kraw[:, sl], start=True, stop=True)
                    t1 = tmp.tile([P, 512], f32, tag="t1")
                    nc.vector.tensor_mul(t1[:], kraw[:, sl], cosk[:, sl])
                    t2 = tmp.tile([P, 512], f32, tag="t2")
                    nc.vector.tensor_mul(t2[:], psw[:], sink[:, sl])
                    nc.vector.tensor_add(krot[:, sl], t1[:], t2[:])
                # --- v_aug block: 16 tiles [128 tok, 2, 65] (two heads x 65) ---
                vtiles = []
                for mt in range(NKT):
                    vt = vstream.tile([P, 2, 65], bf16, tag=f"vaug{mt}")
                    nc.vector.memset(vt[:, :, 64], 1.0)
                    ps = psA.tile([P, 512], f32, tag="ps")
                    for k in range(KD):
                        nc.tensor.matmul(ps[:, 0:P], xkv[k][:, mt * P:(mt + 1) * P],
                                         wv[k][:, c0:c0 + P],
                                         start=(k == 0), stop=(k == KD - 1))
                    # psum [128 tok, 128 feats] -> cols 0:64 head even, 65:129 head odd
                    nc.vector.tensor_copy(vt[:, :, 0:64],
                                          ps[:, 0:P].rearrange("p (h d) -> p h d", h=2))
                    vtiles.append(vt)
                state[hp] = (qrot, krot, vtiles)

            def emit_attn(hp):
                qrot, krot, vtiles = state.pop(hp)
                for half in range(2):
                    hoff = half * DH
                    # QK^T + exp: S^T tiles [128 kj, NQ]
                    pts = []
                    for kt in range(NKT):
                        ps = psS.tile([P, NQ], f32, tag="s")
                        for qn in range(NQ // 512):
                            nc.tensor.matmul(
                                ps[:, qn * 512:(qn + 1) * 512],
                                krot[hoff:hoff + DH, kt * P:(kt + 1) * P],
                                qrot[hoff:hoff + DH, qn * 512:(qn + 1) * 512],
                                start=True, stop=True)
                        pt = ptp.tile([P, NQ], bf16, tag="pt")
                        nc.scalar.activation(pt[:], ps[:], Exp)
                        pts.append(pt)
                    # AV (M=65, ones-column gives denominators in row 64)
                    pvs = [psV.tile([65, 512], f32, tag="av", name="av") for _ in range(NQ // 512)]
                    for kt in range(NKT):
                        for qn in range(NQ // 512):
                            nc.tensor.matmul(pvs[qn][:], vtiles[kt][:, half, :],
                                             pts[kt][:, qn * 512:(qn + 1) * 512],
                                             start=(kt == 0), stop=(kt == NKT - 1))
                    for qn in range(NQ // 512):
                        sl = slice(qn * 512, (qn + 1) * 512)
                        pv = pvs[qn]
                        rec = small.tile([1, 512], f32, tag="rec")
                        nc.vector.reciprocal(rec[:], pv[64:65, :])
                        recb = small.tile([64, 512], f32, tag="recb")
                        nc.gpsimd.partition_broadcast(recb[:], rec[:])
                        nc.vector.tensor_mul(attnT[hp][hoff:hoff + DH, sl],
                                             pv[0:64, :], recb[:])

            for hp in range(HPB + 1):
                if hp < HPB:
                    emit_proj(hp)
                if hp >= 1:
                    emit_attn(hp - 1)

            # ---- out projection: out[tok, DIM] = attnT.T @ Wout ----
            for mt in range(NQ // P):
                for n in range(DIM // 512):
                    ps = psA.tile([P, 512], f32, tag="ps")
                    for k in range(KD):
                        nc.tensor.matmul(ps[:], attnT[k][:, mt * P:(mt + 1) * P],
                                         wo[k][:, n * 512:(n + 1) * 512],
                                         start=(k == 0), stop=(k == KD - 1))
                    st = ostage.tile([P, 512], f32, tag="ost")
                    nc.scalar.copy(st[:], ps[:])
                    nc.sync.dma_start(
                        out_d[mt * P:(mt + 1) * P, n * 512:(n + 1) * 512], st[:])

    nc.compile()
    return nc


def _prep_inputs(x, sin, cos, Wqkv, Wout):
    """Host-side sharding/layout prep. Returns in_maps list for 8 cores."""
    x = np.asarray(x, np.float32)
    Wqkv = np.asarray(Wqkv, np.float32)
    Wout = np.asarray(Wout, np.float32)
    scale = DH ** -0.5
    wq = (Wqkv[:, :INNER] * scale).astype(BF)
    wk = Wqkv[:, INNER:2 * INNER].astype(BF)
    wv = Wqkv[:, 2 * INNER:].astype(BF)
    wo = Wout.astype(BF)
    cos_pad, sin_pad, Rm = _build_rope_consts()
    rm = Rm.astype(BF)

    in_maps = []
    for c in range(NCORES):
        b, half = divmod(c, 2)
        xT = np.ascontiguousarray(x[b].T)                          # [DIM, N]
        ck, sk = cos_pad, sin_pad
        if half == 1:        # rotate tokens so this core's queries come first
            xT = np.concatenate([xT[:, NQ:], xT[:, :NQ]], axis=1)
            ck = np.concatenate([ck[:, NQ:], ck[:, :NQ]], axis=1)
            sk = np.concatenate([sk[:, NQ:], sk[:, :NQ]], axis=1)
        in_maps.append({
            "xkv": np.ascontiguousarray(xT).astype(BF),
            "wq": wq, "wk": wk, "wv": wv, "wo": wo,
            "cosk": np.ascontiguousarray(ck).astype(BF),
            "sink": np.ascontiguousarray(sk).astype(BF),
            "rm": rm,
        })
    return in_maps


LAST_RESULTS = None


def kernel(x, sin, cos, Wqkv, Wout):
    global LAST_RESULTS
    if "nc" not in _CACHE:
        _CACHE["nc"] = _build_program()
    nc = _CACHE["nc"]
    in_maps = _prep_inputs(x, sin, cos, Wqkv, Wout)
    trace = bool(int(os.environ.get("KERNEL_TRACE", "0")))
    res = run_bass_kernel_spmd(nc, in_maps, core_ids=list(range(NCORES)),
                               trace=trace)
    LAST_RESULTS = res
    out = np.empty((B, N, DIM), np.float32)
    for c in range(NCORES):
        b, half = divmod(c, 2)
        out[b, half * NQ:(half + 1) * NQ, :] = res.results[c]["out"]
    return out
